# revision 6
# baseline (speedup 1.0000x reference)
"""Guided filter (radius=3) on 8x TRN2 NeuronCores, batch-parallel.

Per core: one image. Box filters = banded matmuls on the PE:
  pass A1: lhsT = image block (stationary), rhs = vertical band -> (w, h') transposed, PSUM-accumulated
  pass A2: lhsT = horizontal band (stationary), rhs = A1 evac    -> (w', h') natural-per-axis
Stage-2 (boxes of a, b) repeats the pair, returning to natural layout.
Band weights are exactly 1/8 (bf16-exact); the 64/49 normalization is folded
into ScalarE evac copies so box results stay exact-scale in fp32 PSUM.
"""

import sys

sys.path.insert(0, "/opt/trn_rl_repo")

import numpy as np
import ml_dtypes

R = 3
H = W = 1024
P = 128
NC_N = 8
V = 122  # valid outputs per 128-wide band matmul
S = float(64.0 / 49.0)

_cache = {}


def _strips():
    # (in_lo, in_hi, out_lo, out_hi) along one axis
    out = []
    j = 0
    while j * V < W:
        o_lo, o_hi = j * V, min(W, j * V + V)
        i_lo, i_hi = max(0, o_lo - R), min(W, o_hi + R)
        out.append((i_lo, i_hi, o_lo, o_hi))
        j += 1
    return out


def _band7_np():
    b = np.zeros((128, 134), np.float32)
    for k in range(128):
        for d in range(134):
            if abs(d - 3 - k) <= R:
                b[k, d] = 0.125
    return b.astype(ml_dtypes.bfloat16)


def _bandm_np(i_lo, i_hi, o_lo, o_hi):
    K = i_hi - i_lo
    bm = np.zeros((K, 128), np.float32)
    for k in range(K):
        for m in range(o_hi - o_lo):
            if abs((i_lo + k) - (o_lo + m)) <= R:
                bm[k, m] = 0.125
    return bm.astype(ml_dtypes.bfloat16)


def _seg512(lo, hi):
    """split [lo,hi) at multiples of 512 (PSUM bank boundaries)"""
    segs = []
    while lo < hi:
        nxt = min(hi, (lo // 512 + 1) * 512)
        segs.append((lo, nxt))
        lo = nxt
    return segs


def _build():
    import concourse.bass as bass
    import concourse.bacc as bacc
    import concourse.mybir as mybir
    from concourse import tile

    bf16 = mybir.dt.bfloat16
    f32 = mybir.dt.float32
    Copy = mybir.ActivationFunctionType.Copy
    Alu = mybir.AluOpType

    strips = _strips()
    NS = len(strips)

    nc = bacc.Bacc(None, target_bir_lowering=False)
    dI = nc.dram_tensor("I", [H, W], f32, kind="ExternalInput")
    dp = nc.dram_tensor("p", [3, H, W], f32, kind="ExternalInput")
    db7 = nc.dram_tensor("band7", [128, 134], bf16, kind="ExternalInput")
    dbm_f = nc.dram_tensor("bandm_first", [125, 128], bf16, kind="ExternalInput")
    dbm_i = nc.dram_tensor("bandm_int", [128, 128], bf16, kind="ExternalInput")
    dbm_l = nc.dram_tensor("bandm_last", [51, 128], bf16, kind="ExternalInput")
    dq = nc.dram_tensor("q", [3, H, W], f32, kind="ExternalOutput")

    with tile.TileContext(nc) as tc:
        with (
            tc.tile_pool(name="const", bufs=1) as constp,
            tc.tile_pool(name="inp", bufs=2) as inp,
            tc.tile_pool(name="prod", bufs=2) as prodp,
            tc.tile_pool(name="vt", bufs=3) as vtp,
            tc.tile_pool(name="mean", bufs=2) as meanp,
            tc.tile_pool(name="f32t", bufs=1) as f32p,
            tc.tile_pool(name="ab", bufs=1) as abp,
            tc.tile_pool(name="fin", bufs=2) as finp,
            tc.tile_pool(name="psA", bufs=2, space="PSUM") as psA,
            tc.tile_pool(name="psB", bufs=2, space="PSUM") as psB,
        ):
            band7 = constp.tile([128, 134], bf16, tag="band7")
            nc.sync.dma_start(band7[:], db7.ap()[:])
            bm_first = constp.tile([125, 128], bf16, tag="bmf")
            nc.sync.dma_start(bm_first[:], dbm_f.ap()[:])
            bm_int = constp.tile([128, 128], bf16, tag="bmi")
            nc.sync.dma_start(bm_int[:], dbm_i.ap()[:])
            bm_last = constp.tile([51, 128], bf16, tag="bml")
            nc.sync.dma_start(bm_last[:], dbm_l.ap()[:])

            def bandm_for(si):
                if si == 0:
                    return bm_first
                if si == NS - 1:
                    return bm_last
                return bm_int

            def a1_pass(ps, lhs_tile, Mw, dst_valid=1024):
                """vertical box + transpose: accumulate 8 h-blocks into ps[0:Mw, 0:1024].

                start=True clears has_written for the WHOLE psum bank, so only
                the first matmul touching each 512-wide bank may use it; all
                later matmuls accumulate (has_written=0 regions are overwritten).
                """
                seen = set()
                for i in range(8):
                    lhsT = lhs_tile[:, i * Mw : (i + 1) * Mw]
                    base = 128 * i - 3
                    f_lo = 0 if i == 0 else 128 * i + 3
                    f_hi = min(1024, 128 * i + 131)
                    wins = [(f_lo, f_hi)]
                    if i > 0:
                        wins.append((128 * i - 3, 128 * i + 3))
                    for w_lo_, w_hi_ in wins:
                        for s_lo, s_hi in _seg512(w_lo_, w_hi_):
                            bank = s_lo // 512
                            nc.tensor.matmul(
                                ps[0:Mw, s_lo:s_hi],
                                lhsT,
                                band7[:, s_lo - base : s_hi - base],
                                start=bank not in seen,
                                stop=True,
                            )
                            seen.add(bank)

            def a2_pass(ps, vt_tile, si):
                """horizontal box via band-stationary matmul: ps[0:128, 0:1024]"""
                i_lo, i_hi, o_lo, o_hi = strips[si]
                K = i_hi - i_lo
                bm = bandm_for(si)
                for s_lo, s_hi in _seg512(0, 1024):
                    nc.tensor.matmul(
                        ps[:, s_lo:s_hi],
                        bm[:],
                        vt_tile[0:K, s_lo:s_hi],
                        start=True,
                        stop=True,
                    )

            # ---------------- phase 1: stage-1 stats -> a, b (bf16, (w',h') layout)
            ab_tiles = {}
            for si, (i_lo, i_hi, o_lo, o_hi) in enumerate(strips):
                Mw = i_hi - i_lo
                K_out = o_hi - o_lo
                iw = inp.tile([128, 8 * Mw], bf16, tag="iw")
                nc.gpsimd.dma_start(
                    iw[:].rearrange("p (i w) -> p i w", w=Mw),
                    dI.ap()[:, i_lo:i_hi].rearrange("(i p) w -> p i w", p=128),
                )
                ii = prodp.tile([128, 8 * Mw], bf16, tag="ii")
                nc.vector.tensor_mul(ii[:], iw[:], iw[:])

                # box I
                psa = psA.tile([128, 1024], f32, tag="psa")
                a1_pass(psa, iw, Mw)
                vt = vtp.tile([128, 1024], bf16, tag="vt")
                nc.vector.tensor_copy(vt[0:Mw, :], psa[0:Mw, :])
                psb = psB.tile([128, 1024], f32, tag="psb")
                a2_pass(psb, vt, si)
                uI = meanp.tile([128, 1024], bf16, tag="uI")
                nc.scalar.activation(uI[0:K_out, :], psb[0:K_out, :], Copy, bias=0.0, scale=S)

                # box II
                psa2 = psA.tile([128, 1024], f32, tag="psa")
                a1_pass(psa2, ii, Mw)
                vt2 = vtp.tile([128, 1024], bf16, tag="vt")
                nc.vector.tensor_copy(vt2[0:Mw, :], psa2[0:Mw, :])
                psb2 = psB.tile([128, 1024], f32, tag="psb")
                a2_pass(psb2, vt2, si)
                uII = meanp.tile([128, 1024], bf16, tag="uII")
                nc.scalar.activation(uII[0:K_out, :], psb2[0:K_out, :], Copy, bias=0.0, scale=S)

                # rv = 1/var
                sq = meanp.tile([128, 1024], bf16, tag="tmp")
                nc.vector.tensor_mul(sq[0:K_out, :], uI[0:K_out, :], uI[0:K_out, :])
                var_e = f32p.tile([128, 1024], f32, tag="var")
                nc.vector.tensor_sub(var_e[0:K_out, :], uII[0:K_out, :], sq[0:K_out, :])
                rv = f32p.tile([128, 1024], f32, tag="rv")
                nc.vector.reciprocal_approx_fast(rv[0:K_out, :], var_e[0:K_out, :])
                rv_bf = meanp.tile([128, 1024], bf16, tag="rvbf")
                nc.scalar.activation(rv_bf[0:K_out, :], rv[0:K_out, :], Copy, bias=0.0, scale=1.0)

                for c in range(3):
                    pw = inp.tile([128, 8 * Mw], bf16, tag="pw")
                    nc.gpsimd.dma_start(
                        pw[:].rearrange("p (i w) -> p i w", w=Mw),
                        dp.ap()[c][:, i_lo:i_hi].rearrange("(i p) w -> p i w", p=128),
                    )
                    ip = prodp.tile([128, 8 * Mw], bf16, tag="ip")
                    nc.vector.tensor_mul(ip[:], iw[:], pw[:])

                    psa3 = psA.tile([128, 1024], f32, tag="psa")
                    a1_pass(psa3, pw, Mw)
                    vt3 = vtp.tile([128, 1024], bf16, tag="vt")
                    nc.vector.tensor_copy(vt3[0:Mw, :], psa3[0:Mw, :])
                    psb3 = psB.tile([128, 1024], f32, tag="psb")
                    a2_pass(psb3, vt3, si)
                    up = meanp.tile([128, 1024], bf16, tag="up")
                    nc.scalar.activation(up[0:K_out, :], psb3[0:K_out, :], Copy, bias=0.0, scale=S)

                    psa4 = psA.tile([128, 1024], f32, tag="psa")
                    a1_pass(psa4, ip, Mw)
                    vt4 = vtp.tile([128, 1024], bf16, tag="vt")
                    nc.vector.tensor_copy(vt4[0:Mw, :], psa4[0:Mw, :])
                    psb4 = psB.tile([128, 1024], f32, tag="psb")
                    a2_pass(psb4, vt4, si)
                    uIp = meanp.tile([128, 1024], bf16, tag="uIp")
                    nc.scalar.activation(uIp[0:K_out, :], psb4[0:K_out, :], Copy, bias=0.0, scale=S)

                    w_t = meanp.tile([128, 1024], bf16, tag="tmp")
                    nc.vector.tensor_mul(w_t[0:K_out, :], uI[0:K_out, :], up[0:K_out, :])
                    cov = meanp.tile([128, 1024], bf16, tag="tmp")
                    nc.vector.tensor_sub(cov[0:K_out, :], uIp[0:K_out, :], w_t[0:K_out, :])
                    a_t = abp.tile([128, 1024], bf16, tag=f"a_{c}_{si}")
                    nc.vector.tensor_mul(a_t[0:K_out, :], cov[0:K_out, :], rv_bf[0:K_out, :])
                    t3 = meanp.tile([128, 1024], bf16, tag="tmp")
                    nc.vector.tensor_mul(t3[0:K_out, :], a_t[0:K_out, :], uI[0:K_out, :])
                    b_t = abp.tile([128, 1024], bf16, tag=f"b_{c}_{si}")
                    nc.vector.tensor_sub(b_t[0:K_out, :], up[0:K_out, :], t3[0:K_out, :])
                    ab_tiles[("a", c, si)] = a_t
                    ab_tiles[("b", c, si)] = b_t

            # ---------------- phase 2: box a, b; combine into q
            def b1_pass(ps, which, c, m_lo, m_hi):
                """H-box of a/b over w'-strips; out ps[0:(m_hi-m_lo), 0:1024] (h' window on partitions)"""
                seen = set()
                for sj, (ji_lo, ji_hi, jo_lo, jo_hi) in enumerate(strips):
                    K = jo_hi - jo_lo
                    t = ab_tiles[(which, c, sj)]
                    lhsT = t[0:K, m_lo:m_hi]
                    base = jo_lo - 3
                    f_lo = 0 if sj == 0 else jo_lo + 3
                    f_hi = min(1024, jo_lo + 125)
                    wins = [(f_lo, f_hi)]
                    if sj > 0:
                        wins.append((jo_lo - 3, jo_lo + 3))
                    for w_lo_, w_hi_ in wins:
                        for s_lo, s_hi in _seg512(w_lo_, w_hi_):
                            bank = s_lo // 512
                            nc.tensor.matmul(
                                ps[0 : m_hi - m_lo, s_lo:s_hi],
                                lhsT,
                                band7[0:K, s_lo - base : s_hi - base],
                                start=bank not in seen,
                                stop=True,
                            )
                            seen.add(bank)

            for m, (mi_lo, mi_hi, mo_lo, mo_hi) in enumerate(strips):
                Hw = mo_hi - mo_lo
                ifull = finp.tile([128, 1024], f32, tag="ifull")
                nc.sync.dma_start(ifull[0:Hw, :], dI.ap()[mo_lo:mo_hi, :])
                for c in range(3):
                    psc_a = psA.tile([128, 1024], f32, tag="psa")
                    b1_pass(psc_a, "a", c, mi_lo, mi_hi)
                    abox = vtp.tile([128, 1024], bf16, tag="vt")
                    nc.scalar.activation(
                        abox[0 : mi_hi - mi_lo, :], psc_a[0 : mi_hi - mi_lo, :], Copy, bias=0.0, scale=S
                    )
                    psc_b = psA.tile([128, 1024], f32, tag="psa")
                    b1_pass(psc_b, "b", c, mi_lo, mi_hi)
                    bbox = vtp.tile([128, 1024], bf16, tag="vt")
                    nc.scalar.activation(
                        bbox[0 : mi_hi - mi_lo, :], psc_b[0 : mi_hi - mi_lo, :], Copy, bias=0.0, scale=S
                    )
                    psd_a = psB.tile([128, 1024], f32, tag="psb")
                    a2_pass(psd_a, abox, m)
                    psd_b = psB.tile([128, 1024], f32, tag="psb")
                    a2_pass(psd_b, bbox, m)
                    t_t = finp.tile([128, 1024], f32, tag="tt")
                    nc.vector.tensor_mul(t_t[0:Hw, :], psd_a[0:Hw, :], ifull[0:Hw, :])
                    nc.vector.tensor_add(t_t[0:Hw, :], t_t[0:Hw, :], psd_b[0:Hw, :])
                    nc.vector.tensor_scalar(
                        t_t[0:Hw, :], t_t[0:Hw, :], 1.0, 0.0, Alu.min, Alu.max
                    )
                    nc.sync.dma_start(dq.ap()[c][mo_lo:mo_hi, :], t_t[0:Hw, :])

    nc.compile()
    return nc


def kernel(I, p, radius):
    assert int(radius) == R
    I = np.ascontiguousarray(np.asarray(I, np.float32))
    p = np.ascontiguousarray(np.asarray(p, np.float32))
    B = I.shape[0]
    assert I.shape == (B, 1, H, W) and p.shape == (B, 3, H, W)

    if "nc" not in _cache:
        _cache["nc"] = _build()
    nc = _cache["nc"]

    from concourse.bass_utils import run_bass_kernel_spmd

    b7 = _band7_np()
    strips = _strips()
    bm_f = _bandm_np(*strips[0])
    bm_i = _bandm_np(*strips[1])
    bm_l = _bandm_np(*strips[-1])

    in_maps = []
    for i in range(B):
        in_maps.append(
            {
                "I": I[i, 0],
                "p": p[i],
                "band7": b7,
                "bandm_first": bm_f,
                "bandm_int": bm_i,
                "bandm_last": bm_l,
            }
        )
    res = run_bass_kernel_spmd(nc, in_maps, core_ids=list(range(B)))
    out = np.stack([res.results[i]["q"] for i in range(B)], axis=0)
    return out.astype(np.float32)
